# revision 5
# baseline (speedup 1.0000x reference)
"""Trainium2 Bass kernel for nn_LoRALinear (DoRA-style LoRA linear).

Reference math (per problem):
    base = x @ W^T
    lora = sc * (x @ A^T) @ B^T          (sc = 2.0)
    w_eff = W + sc * (B @ A)
    s = magnitude / ||w_eff||_row         (row norm over in_dim)
    out = base + (s - 1) * base + s * lora
        = s * (base + lora)
        = x @ (s[:, None] * w_eff)^T

The whole op collapses to one dense matmul with a derived weight. The
derived weight is tiny (1024x1024, 0.05% of the FLOPs) and is computed
host-side in fp32 during input prep (the same place the shards are cut),
so the device kernel is a pure streaming GEMM.

Strategy: data-parallel shard x over batch*seq across 8 cores. Host prep:
  - ws = ((W + 2 B A) * s[:, None])^T as bf16  [d_in, d_out]  (replicated)
  - xT = x-shard^T as bf16                     [d_in, 4096]   (per core)

Per-core device kernel (pure bf16 matmul, fp32 PSUM accumulate), tuned
for the two HWDGE descriptor-gen queues (~4.75ns/SBUF-line each, so
engine assignment matters as much as bytes):
  - weight tiles DMA'd on the ACT queue (free during the Sync-side entry
    preamble), x chunks on the Sync queue: first matmul starts ~2us
    earlier than a single-queue issue order
  - x streamed in 4 chunks of 1024 tokens (2KB DMA lines, halves x
    descriptor-gen vs 256-token chunks)
  - psum waves of 256 tokens: 4 banks per wave (2 row-groups x 2
    n-halves), tags double-buffered -> all 8 banks, PE never waits
  - k-outer accumulation (for k, for (jj,h): matmul, start=k==0,
    stop=k==7) so the k=0 stage only needs the first weight tile at
    startup instead of all 8
  - psum drains split: n-half 0 on ACT, n-half 1 on DVE; out DMA
    triggered from ACT. Per 1024-token chunk: Sync 5.2us, ACT 10.9us,
    DVE 5.7us, PE 27.3us -- PE is the only saturated engine.
Host converts the bf16 output back to fp32. bf16 keeps relative error
~3e-3, well under the 2e-2 gate.
"""

import os
import numpy as np
from contextlib import ExitStack

import ml_dtypes

import concourse.bass as bass
import concourse.mybir as mybir
import concourse.tile as tile
from concourse import bacc
from concourse.bass import ts
from concourse.bass_utils import run_bass_kernel_spmd

N_CORES = 8
B, S, D_IN, D_OUT, R = 4, 8192, 1024, 1024, 16
SCALING = 32.0 / 16.0
M_TOT = B * S                 # 32768 tokens
M_CORE = M_TOT // N_CORES     # 4096 tokens per core
P = 128
K_TILES = D_IN // P           # 8
CHUNK = 1024                  # tokens per x DMA chunk
N_CHUNKS = M_CORE // CHUNK    # 4
WAVE = 256                    # tokens per psum wave
WAVES = CHUNK // WAVE         # 4
SUB = WAVE // P               # 2 psum row-groups per wave
NH = D_OUT // 512             # 2 n-halves of 512
F32 = mybir.dt.float32
BF16 = mybir.dt.bfloat16
BF16_NP = np.dtype(ml_dtypes.bfloat16)


def _kernel_body(ctx: ExitStack, tc: "tile.TileContext", xT, wsT, out):
    nc = tc.nc
    w_pool = ctx.enter_context(tc.tile_pool(name="w", bufs=1))
    x_pool = ctx.enter_context(tc.tile_pool(name="x", bufs=3))
    o_pool = ctx.enter_context(tc.tile_pool(name="o", bufs=4))
    ps_pool = ctx.enter_context(tc.tile_pool(name="ps", bufs=2, space="PSUM"))

    # Weights on the (otherwise idle) GpSimd software-DGE queue so their
    # descriptor-gen runs concurrently with the x triggers on Sync; the
    # k-outer matmul order below only needs w[k] at stage k, so the PE
    # starts after ~1 weight tile instead of all 8.
    ws = []
    for k in range(K_TILES):
        w = w_pool.tile([P, D_OUT], BF16, tag=f"w{k}", name=f"w{k}")
        nc.gpsimd.dma_start(w[:], wsT[ts(k, P), :])
        ws.append(w)

    def load_chunk(c):
        xts = []
        for k in range(K_TILES):
            xt = x_pool.tile([P, CHUNK], BF16, tag=f"xt{k}", name=f"xt{k}_{c}")
            nc.sync.dma_start(xt[:], xT[ts(k, P), ts(c, CHUNK)])
            xts.append(xt)
        return xts

    for c in range(N_CHUNKS):
        xts = load_chunk(c)
        for wv in range(WAVES):
            pss = [
                [
                    ps_pool.tile(
                        [P, 512], F32, tag=f"ps{jj}{h}", name=f"ps{jj}{h}_{c}_{wv}"
                    )
                    for h in range(NH)
                ]
                for jj in range(SUB)
            ]
            for k in range(K_TILES):
                for jj in range(SUB):
                    for h in range(NH):
                        nc.tensor.matmul(
                            pss[jj][h][:],
                            lhsT=xts[k][:, ts(wv * SUB + jj, P)],
                            rhs=ws[k][:, ts(h, 512)],
                            start=(k == 0),
                            stop=(k == K_TILES - 1),
                        )
            for jj in range(SUB):
                o_sb = o_pool.tile(
                    [P, D_OUT], BF16, tag=f"o{jj}", name=f"o{jj}_{c}_{wv}"
                )
                nc.scalar.copy(o_sb[:, ts(0, 512)], pss[jj][0][:])
                nc.vector.tensor_copy(o_sb[:, ts(1, 512)], pss[jj][1][:])
                # alternate HWDGE rings so back-to-back out transfers overlap
                eng = nc.scalar if jj == 0 else nc.sync
                eng.dma_start(
                    out[ts(c * WAVES * SUB + wv * SUB + jj, P), :], o_sb[:]
                )


def build_nc() -> "bass.Bass":
    nc = bacc.Bacc(
        "TRN2",
        target_bir_lowering=False,
        debug=False,
        num_devices=N_CORES,
    )
    xT = nc.dram_tensor("xT", [D_IN, M_CORE], BF16, kind="ExternalInput").ap()
    wsT = nc.dram_tensor("wsT", [D_IN, D_OUT], BF16, kind="ExternalInput").ap()
    out = nc.dram_tensor("out", [M_CORE, D_OUT], BF16, kind="ExternalOutput").ap()

    with tile.TileContext(nc) as tc, ExitStack() as ctx:
        _kernel_body(ctx, tc, xT, wsT, out)
    nc.compile()
    return nc


_NC_CACHE: list = []


def get_nc() -> "bass.Bass":
    if not _NC_CACHE:
        _NC_CACHE.append(build_nc())
    return _NC_CACHE[0]


def make_in_maps(x, weight, a_w, b_w, magnitude):
    # Derived DoRA weight, computed in fp32 exactly as the reference does.
    w_eff = weight.astype(np.float32) + np.float32(SCALING) * (
        b_w.astype(np.float32) @ a_w.astype(np.float32)
    )
    norm = np.sqrt((w_eff.astype(np.float64) ** 2).sum(axis=1))
    s = (magnitude.astype(np.float64).reshape(-1) / norm).astype(np.float32)
    wsT = np.ascontiguousarray((w_eff * s[:, None]).T).astype(BF16_NP)

    xb = x.reshape(N_CORES, M_CORE, D_IN).astype(BF16_NP)
    xT = np.ascontiguousarray(np.transpose(xb, (0, 2, 1)))  # [8, d_in, m_core]
    return [{"xT": xT[i], "wsT": wsT} for i in range(N_CORES)]


def kernel(x, weight, a_w, b_w, magnitude):
    nc = get_nc()
    in_maps = make_in_maps(x, weight, a_w, b_w, magnitude)
    trace = os.environ.get("KERNEL_TRACE", "0") == "1"
    res = run_bass_kernel_spmd(nc, in_maps, list(range(N_CORES)), trace=trace)
    if trace:
        kernel.last_result = res
    outs = [res.results[i]["out"] for i in range(N_CORES)]
    return (
        np.concatenate(outs, axis=0).astype(np.float32).reshape(B, S, D_OUT)
    )
